# revision 25
# baseline (speedup 1.0000x reference)
"""HGAT model kernel for 8x Trainium2 NeuronCores.

Structure: 2-layer GRU (T=60, H=128) data-parallel over N=8192 nodes
(1024/core), software-pipelined so the tensor engine never idles long
enough to HAM-throttle; hypergraph attention collapsed through the E=30
hyperedge dim with a [E,1] vector AllReduce (s2) plus a [E,H] AllReduce.
"""

import sys

sys.path.insert(0, "/opt/trn_rl_repo")

import os
import numpy as np
import ml_dtypes

import concourse.bacc as bacc
import concourse.tile as tile
import concourse.mybir as mybir
from concourse.bass_utils import run_bass_kernel_spmd

F32 = mybir.dt.float32
BF16 = mybir.dt.bfloat16
AF = mybir.ActivationFunctionType
ALU = mybir.AluOpType
AX = mybir.AxisListType

N = 8192
T = int(os.environ.get("KERNEL_T", "60"))
DF = 6
H = 128
E = 30
NC = 8
NL = N // NC          # 1024 nodes per core
NCH = NL // 128       # 8 chunks of 128 nodes
SLOPE = 0.01
XQ = 4                # x streamed in 4 quarter-chunks
TQ = (T + XQ - 1) // XQ

_CACHE = {}

HALVES = (slice(0, 512), slice(512, 1024))


def _build_program():
    nc = bacc.Bacc("TRN2", target_bir_lowering=False, debug=False, num_devices=NC)

    dt = BF16

    # ---- DRAM I/O ----
    x_d = nc.dram_tensor("x", [128, T * (NL // 2)], dt, kind="ExternalInput")
    whhT0_d = nc.dram_tensor("whhT0", [H, 3 * H], dt, kind="ExternalInput")
    wihT0_d = nc.dram_tensor("wihT0", [128, 3 * H], dt, kind="ExternalInput")
    whhT1_d = nc.dram_tensor("whhT1", [H, 3 * H], dt, kind="ExternalInput")
    wihT1_d = nc.dram_tensor("wihT1", [H, 3 * H], dt, kind="ExternalInput")
    bias_d = nc.dram_tensor("bias", [H, 8], F32, kind="ExternalInput")
    v12_d = nc.dram_tensor("v12", [H, 2], dt, kind="ExternalInput")
    wfcT_d = nc.dram_tensor("wfcT", [H, H], dt, kind="ExternalInput")
    wout_d = nc.dram_tensor("wout", [H, 1], dt, kind="ExternalInput")
    identd_d = nc.dram_tensor("identd", [H, H], dt, kind="ExternalInput")
    identf_d = nc.dram_tensor("identf", [H, H], F32, kind="ExternalInput")
    gh_d = nc.dram_tensor("gh", [128, NCH * E], F32, kind="ExternalInput")
    invdv_d = nc.dram_tensor("invdv", [128, NCH], F32, kind="ExternalInput")
    invde_d = nc.dram_tensor("invde", [E, 1], F32, kind="ExternalInput")
    ones_d = nc.dram_tensor("ones1", [1, H], F32, kind="ExternalInput")
    y_d = nc.dram_tensor("y", [1, NL], F32, kind="ExternalOutput")

    with tile.TileContext(nc) as tc:
        with (
            tc.tile_pool(name="const", bufs=1) as cp,
            tc.tile_pool(name="xp", bufs=2) as xp,
            tc.tile_pool(name="hp", bufs=2) as hp,
            tc.tile_pool(name="wk", bufs=2) as wk,
            tc.tile_pool(name="pbs", bufs=1) as pbs,
            tc.tile_pool(name="dram", bufs=1, space="DRAM") as dp,
        ):
            # ---- load constants ----
            def cload(dram, shape, dtype):
                t_ = cp.tile(shape, dtype, tag=dram.name)
                nc.sync.dma_start(t_[:], dram[:])
                return t_

            whhT0 = cload(whhT0_d, [H, 3 * H], dt)
            wihT0 = cload(wihT0_d, [128, 3 * H], dt)
            whhT1 = cload(whhT1_d, [H, 3 * H], dt)
            wihT1 = cload(wihT1_d, [H, 3 * H], dt)
            bias = cload(bias_d, [H, 8], F32)
            v12 = cload(v12_d, [H, 2], dt)
            wfcT = cload(wfcT_d, [H, H], dt)
            wout = cload(wout_d, [H, 1], dt)
            identd = cload(identd_d, [H, H], dt)
            identf = cload(identf_d, [H, H], F32)
            gh = cload(gh_d, [128, NCH * E], F32)
            invdv = cload(invdv_d, [128, NCH], F32)
            invde = cload(invde_d, [E, 1], F32)
            ones1 = cload(ones_d, [1, H], F32)

            # ---- dummy AllReduce to warm the CC stream (overlaps early GRU) ----
            dum_in = dp.tile([1, 4], F32, tag="dum_in")
            dum_out = dp.tile([1, 4], F32, tag="dum_out")
            nc.sync.dma_start(dum_in[:], bias[0:1, 0:4])
            nc.gpsimd.collective_compute(
                "AllReduce", ALU.add,
                replica_groups=[list(range(NC))],
                ins=[dum_in.opt()], outs=[dum_out.opt()],
            )

            # gate column ranges in the 3H weight layout
            R, Z, G = slice(0, H), slice(H, 2 * H), slice(2 * H, 3 * H)

            HN = NL // 2  # 512 nodes per tower

            def make_tower(name, hf, po):
                h0z = hp.tile([H, HN], dt, tag="h0" + name)
                h1z = hp.tile([H, HN], dt, tag="h1" + name)
                nc.vector.memzero(h0z[:])
                nc.vector.memzero(h1z[:])
                return {"n": name, "hf": hf, "po": po, "h0": h0z, "h1": h1z, "l1": {}}

            twA = make_tower("a", HALVES[0], 0)
            twB = make_tower("b", HALVES[1], 64)
            st = {"xq": None}

            def xt_ap(t, po):
                # x slice for step t at row-tile partition offset po
                q = t // TQ
                off = (t % TQ) * HN
                return st["xq"][po : po + 7, off : off + HN]

            with tc.tile_pool(name="psA", bufs=1, space="PSUM") as psA:
                # per-tower psum: prz [128,1024] ([r|z], 2 banks), pr1/pz1 1 bank each

                def wt(tw, shape, tag):
                    return wk.tile(shape, dt, tag=tag + tw["n"], name=tag + tw["n"])

                def l1_mid(tw, t):
                    # wih1 parts (stop groups) + r/z sigmoids for L1 step t-1
                    l1 = tw["l1"]
                    pr1, pz1 = l1["pr1"], l1["pz1"]
                    nc.tensor.matmul(pr1[:], wihT1[:, R], tw["h0"][:],
                                     start=False, stop=True)
                    nc.tensor.matmul(pz1[:], wihT1[:, Z], tw["h0"][:],
                                     start=False, stop=True)
                    r1 = wt(tw, [H, HN], "r1")
                    nc.scalar.activation(r1[:], pr1[:], AF.Sigmoid, bias=bias[:, 0:1])
                    z1 = wt(tw, [H, HN], "z1")
                    nc.scalar.activation(z1[:], pz1[:], AF.Sigmoid, bias=bias[:, 1:2])
                    l1["r1"], l1["z1"] = r1, z1

                def l0_mm(tw, t):
                    prz = psA.tile([H, NL], F32, tag="prz" + tw["n"])
                    rz0 = wt(tw, [H, NL], "rz0")
                    po = tw["po"]
                    nc.tensor.matmul(prz[:, 0:HN], wihT0[po : po + 7, R],
                                     xt_ap(t, po), start=True, stop=False,
                                     tile_position=(po, 0))
                    nc.tensor.matmul(prz[:, HN:NL], wihT0[po + 32 : po + 39, Z],
                                     xt_ap(t, po + 32), start=True, stop=False,
                                     tile_position=(po + 32, 0))
                    nc.tensor.matmul(prz[:, 0:HN], whhT0[:, R], tw["h0"][:],
                                     start=False, stop=True)
                    nc.tensor.matmul(prz[:, HN:NL], whhT0[:, Z], tw["h0"][:],
                                     start=False, stop=True)
                    nc.scalar.activation(rz0[:], prz[:], AF.Sigmoid)
                    tw["prz"], tw["rz0"] = prz, rz0

                def l1_lateA(tw, t):
                    l1 = tw["l1"]
                    pr1, pz1 = l1["pr1"], l1["pz1"]
                    nc.tensor.matmul(pr1[:], whhT1[:, G], tw["h1"][:],
                                     start=True, stop=True)
                    nc.tensor.matmul(pz1[:], wihT1[:, G], tw["h0"][:],
                                     start=True, stop=False)
                    rh1 = wt(tw, [H, HN], "rh1")
                    nc.vector.scalar_tensor_tensor(
                        rh1[:], pr1[:], bias[:, 3:4], l1["r1"][:],
                        ALU.add, ALU.mult)
                    nc.tensor.matmul(pz1[:], identd[:], rh1[:],
                                     start=False, stop=True)

                def l0_stt(tw, t):
                    prz, rz0, po = tw["prz"], tw["rz0"], tw["po"]
                    nc.tensor.matmul(prz[:, 0:HN], whhT0[:, G], tw["h0"][:],
                                     start=True, stop=True)
                    nc.tensor.matmul(prz[:, HN:NL], wihT0[po : po + 7, G],
                                     xt_ap(t, po), start=True, stop=False,
                                     tile_position=(po, 0))
                    rh0 = wt(tw, [H, HN], "rh0")
                    nc.vector.scalar_tensor_tensor(
                        rh0[:], prz[:, 0:HN], bias[:, 2:3], rz0[:, 0:HN],
                        ALU.add, ALU.mult)
                    nc.tensor.matmul(prz[:, HN:NL], identd[:], rh0[:],
                                     start=False, stop=True)

                def l1_lateB(tw, t, ueng):
                    l1 = tw["l1"]
                    pz1 = l1["pz1"]
                    n1 = wt(tw, [H, HN], "n1")
                    nc.scalar.activation(n1[:], pz1[:], AF.Tanh, bias=bias[:, 4:5])
                    d1 = wt(tw, [H, HN], "d1")
                    nc.vector.tensor_sub(d1[:], tw["h1"][:], n1[:])
                    zd1 = wt(tw, [H, HN], "zd1")
                    nc.vector.tensor_mul(zd1[:], l1["z1"][:], d1[:])
                    h1n = hp.tile([H, HN], dt, tag="h1" + tw["n"])
                    nc.vector.tensor_add(h1n[:], n1[:], zd1[:])
                    tw["h1"] = h1n

                def l0_tail(tw, t, ueng):
                    prz, rz0 = tw["prz"], tw["rz0"]
                    n0 = wt(tw, [H, HN], "n0")
                    nc.scalar.activation(n0[:], prz[:, HN:NL], AF.Tanh)
                    d0 = wt(tw, [H, HN], "d0")
                    ueng.tensor_sub(d0[:], tw["h0"][:], n0[:])
                    zd0 = wt(tw, [H, HN], "zd0")
                    nc.vector.tensor_mul(zd0[:], rz0[:, HN:NL], d0[:])
                    h0n = hp.tile([H, HN], dt, tag="h0" + tw["n"])
                    nc.vector.tensor_add(h0n[:], n0[:], zd0[:])
                    tw["h0"] = h0n

                def l1_early(tw, t):
                    l1 = tw["l1"]
                    pr1 = psA.tile([H, HN], F32, tag="pr1" + tw["n"])
                    pz1 = psA.tile([H, HN], F32, tag="pz1" + tw["n"])
                    nc.tensor.matmul(pr1[:], whhT1[:, R], tw["h1"][:],
                                     start=True, stop=False)
                    nc.tensor.matmul(pz1[:], whhT1[:, Z], tw["h1"][:],
                                     start=True, stop=False)
                    l1["pr1"], l1["pz1"] = pr1, pz1

                def emit_l1_final(tw):
                    l1_mid(tw, T)
                    l1_lateA(tw, T)
                    l1_lateB(tw, T, nc.vector)

                def ueng_for(s):
                    return nc.vector if s < 8 else nc.gpsimd

                for t in range(T + 1):
                    if t % TQ == 0 and t < T:
                        xq = xp.tile([128, TQ * HN], dt, tag="xq")
                        nc.sync.dma_start(xq[:], x_d[:, t * HN : (t + TQ) * HN])
                        st["xq"] = xq
                    # tower B runs one step behind A; B's ops lead each block
                    steps = []
                    if t >= 1:
                        steps.append((twB, t - 1))
                    if t < T:
                        steps.append((twA, t))
                    for tw, s in steps:
                        if s > 0:
                            l1_mid(tw, s)
                    for tw, s in steps:
                        l0_mm(tw, s)
                    for tw, s in steps:
                        if s > 0:
                            l1_lateA(tw, s)
                    for tw, s in steps:
                        l0_stt(tw, s)
                    for tw, s in steps:
                        if s > 0:
                            l1_lateB(tw, s, ueng_for(s))
                    for tw, s in steps:
                        l0_tail(tw, s, ueng_for(s))
                    for tw, s in steps:
                        l1_early(tw, s)
                emit_l1_final(twA)
                emit_l1_final(twB)

            h1ab = (twA["h1"], twB["h1"])  # two [H, 512] bf16 tiles

            # ================= attention tail =================
            hid_nm = pbs.tile([128, NL], F32)  # node-major hidden
            s12s = pbs.tile([128, 2 * NCH], F32)
            s2loc = pbs.tile([E, 1], F32)
            attd = pbs.tile([128, NCH * E], F32)
            attdT = pbs.tile([E, NL], F32)
            S_sb = pbs.tile([E, H], F32)

            with tc.tile_pool(name="psT", bufs=1, space="PSUM") as psT:
                ps_fc = psT.tile([128, NL], F32, tag="fc")      # 2 banks
                ps_tr = psT.tile([128, NL], dt, tag="tr")       # 1 bank
                ps_g2 = psT.tile([128, NL], F32, tag="g2")      # 2 banks
                ps_S = psT.tile([E, NL], F32, tag="S")          # 2 banks
                ps_ms = psT.tile([128, 512], F32, tag="misc")   # 1 bank

                def hid_chunk(c):
                    return h1ab[c // 4][:, 128 * (c % 4) : 128 * (c % 4 + 1)]

                # --- phase A: node-major transpose, s1/w2, local s2 ---
                for c in range(NCH):
                    cs = slice(128 * c, 128 * (c + 1))
                    nc.tensor.transpose(ps_tr[:, cs], hid_chunk(c), identd[:])
                nc.scalar.copy(hid_nm[:], ps_tr[:])
                for c in range(NCH):
                    nc.tensor.matmul(
                        ps_fc[:, 2 * c : 2 * c + 2], hid_chunk(c), v12[:],
                        start=True, stop=True,
                    )
                nc.scalar.copy(s12s[:], ps_fc[:, 0 : 2 * NCH])
                for c in range(NCH):
                    nc.tensor.matmul(
                        ps_fc[0:E, 31:32], gh[:, E * c : E * (c + 1)],
                        s12s[:, 2 * c + 1 : 2 * c + 2],
                        start=(c == 0), stop=(c == NCH - 1),
                    )
                nc.scalar.copy(s2loc[:], ps_fc[0:E, 31:32])

                s2_in = dp.tile([E, 1], F32, tag="s2_in")
                s2_out = dp.tile([E, 1], F32, tag="s2_out")
                nc.sync.dma_start(s2_in[:], s2loc[:])
                nc.gpsimd.collective_compute(
                    "AllReduce", ALU.add,
                    replica_groups=[list(range(NC))],
                    ins=[s2_in.opt()], outs=[s2_out.opt()],
                )
                s2g = pbs.tile([E, 1], F32)
                nc.sync.dma_start(s2g[:], s2_out[:])

                # --- phase B (overlaps AR#1): fc partial from hid ---
                for hi, hf in enumerate(HALVES):
                    nc.tensor.matmul(
                        ps_fc[:, hf], wfcT[:], h1ab[hi][:],
                        start=True, stop=False,
                    )

                # prewarm the exp table set while AR#1 is in flight
                exw = pbs.tile([1, 1], F32)
                nc.scalar.activation(exw[:], bias[0:1, 0:1], AF.Exp)

                # --- phase C: scores + softmax + local S ---
                s2c = pbs.tile([E, 1], F32)
                nc.vector.tensor_scalar_add(s2c[:], s2g[:], bias[0:E, 5:6])
                nc.tensor.transpose(ps_ms[0:1, 32:62], s2c[:], identf[0:E, 0:E])
                s2r = pbs.tile([1, E], F32)
                nc.scalar.copy(s2r[:], ps_ms[0:1, 32:62])
                nc.tensor.matmul(
                    ps_ms[:, 0:E], ones1[:], s2r[:], start=True, stop=True
                )
                s2b = pbs.tile([128, E], F32)
                nc.scalar.copy(s2b[:], ps_ms[:, 0:E])

                sc_t = pbs.tile([128, NCH * E], F32)
                lr_t = pbs.tile([128, NCH * E], F32)
                ex_t = pbs.tile([128, NCH * E], F32)
                nmx = pbs.tile([128, NCH], F32)
                se = pbs.tile([128, NCH], F32)
                rs = pbs.tile([128, NCH], F32)
                for c in range(NCH):
                    es = slice(E * c, E * (c + 1))
                    nc.vector.tensor_scalar_add(
                        sc_t[:, es], s2b[:], s12s[:, 2 * c : 2 * c + 1]
                    )
                for c in range(NCH):
                    es = slice(E * c, E * (c + 1))
                    nc.vector.scalar_tensor_tensor(
                        lr_t[:, es], sc_t[:, es], SLOPE, sc_t[:, es],
                        ALU.mult, ALU.max,
                    )
                for c in range(NCH):
                    es = slice(E * c, E * (c + 1))
                    nc.vector.tensor_reduce(
                        nmx[:, c : c + 1], lr_t[:, es], AX.X, ALU.max, negate=True
                    )
                for c in range(NCH):
                    es = slice(E * c, E * (c + 1))
                    nc.scalar.activation(
                        ex_t[:, es], lr_t[:, es], AF.Exp,
                        bias=nmx[:, c : c + 1],
                    )
                for c in range(NCH):
                    es = slice(E * c, E * (c + 1))
                    nc.vector.tensor_reduce(
                        se[:, c : c + 1], ex_t[:, es], AX.X, ALU.add
                    )
                nc.vector.reciprocal(rs[:], se[:])
                for c in range(NCH):
                    es = slice(E * c, E * (c + 1))
                    nc.vector.tensor_scalar(
                        attd[:, es], ex_t[:, es], rs[:, c : c + 1],
                        invdv[:, c : c + 1], ALU.mult, ALU.mult,
                    )
                for c in range(NCH):
                    cs = slice(128 * c, 128 * (c + 1))
                    es = slice(E * c, E * (c + 1))
                    nc.tensor.matmul(
                        ps_S[:, 0:H], attd[:, es], hid_nm[:, cs],
                        start=(c == 0), stop=(c == NCH - 1),
                    )
                nc.scalar.copy(S_sb[:], ps_S[:, 0:H])

                S_in = dp.tile([E, H], F32, tag="S_in")
                S_out = dp.tile([E, H], F32, tag="S_out")
                nc.sync.dma_start(S_in[:], S_sb[:])
                nc.gpsimd.collective_compute(
                    "AllReduce", ALU.add,
                    replica_groups=[list(range(NC))],
                    ins=[S_in.opt()], outs=[S_out.opt()],
                )
                SF = pbs.tile([E, H], F32)
                nc.sync.dma_start(SF[:], S_out[:])

                # --- phase D (overlaps AR#2): attd transposes ---
                for c in range(NCH):
                    nc.tensor.transpose(
                        ps_S[:, 128 * c : 128 * (c + 1)],
                        attd[:, E * c : E * (c + 1)], identf[:],
                    )
                nc.scalar.copy(attdT[:], ps_S[:])

                # --- phase E: g2, fc, out ---
                Sd = pbs.tile([E, H], F32)
                nc.vector.tensor_scalar_mul(Sd[:], SF[:], invde[:])
                for hf in HALVES:
                    nc.tensor.matmul(
                        ps_g2[:, hf], Sd[:], attdT[:, hf],
                        start=True, stop=True,
                    )
                g2s = pbs.tile([128, NL], dt)
                nc.scalar.copy(g2s[:], ps_g2[:])
                for hf in HALVES:
                    nc.tensor.matmul(
                        ps_fc[:, hf], wfcT[:], g2s[:, hf],
                        start=False, stop=True,
                    )
                fcb = pbs.tile([128, NL], F32)
                nc.scalar.activation(fcb[:], ps_fc[:], AF.Identity, bias=bias[:, 6:7])
                fc2 = pbs.tile([128, NL], dt)
                for hf in HALVES:
                    nc.vector.scalar_tensor_tensor(
                        fc2[:, hf], fcb[:, hf], SLOPE, fcb[:, hf],
                        ALU.mult, ALU.max,
                    )
                y_sb = pbs.tile([1, NL], F32)
                for hf in HALVES:
                    nc.tensor.matmul(
                        ps_ms[0:1, 0:512], wout[:], fc2[:, hf],
                        start=True, stop=True,
                    )
                    nc.vector.tensor_scalar_add(
                        y_sb[0:1, hf], ps_ms[0:1, 0:512], bias[0:1, 7:8]
                    )
                nc.sync.dma_start(y_d[:], y_sb[:])

    nc.finalize()
    return nc


def _prep_inputs(x, GH, Wih0, Whh0, bih0, bhh0, Wih1, Whh1, bih1, bhh1,
                 Wt, bt, a, Wfc, bfc, Wout, bout):
    bf = ml_dtypes.bfloat16
    f32 = np.float32

    a1, a2 = a[:H, 0].astype(f32), a[H:, 0].astype(f32)
    v1 = (Wt.T.astype(f32) @ a1).reshape(H, 1)
    v2 = (Wt.T.astype(f32) @ a2).reshape(H, 1)
    c12 = float(bt.astype(f32) @ a1 + bt.astype(f32) @ a2)

    de = GH.astype(f32).sum(axis=0)
    dv = GH.astype(f32).sum(axis=1) / 2.0
    inv_de = np.where(de != 0, 1.0 / np.where(de != 0, de, 1.0), 0.0).astype(f32)
    inv_dv = np.where(dv != 0, 1.0 / np.where(dv != 0, dv, 1.0), 0.0).astype(f32)

    wihT0_aug = np.zeros((7, 3 * H), f32)
    wihT0_aug[:6] = Wih0.T
    wihT0_aug[6, 0:H] = bih0[0:H] + bhh0[0:H]
    wihT0_aug[6, H : 2 * H] = bih0[H : 2 * H] + bhh0[H : 2 * H]
    wihT0_aug[6, 2 * H :] = bih0[2 * H :]
    wihT0_4x = np.zeros((128, 3 * H), f32)
    for po in (0, 32, 64, 96):
        wihT0_4x[po : po + 7] = wihT0_aug

    bias = np.zeros((H, 8), f32)
    bias[:, 0] = bih1[0:H] + bhh1[0:H]
    bias[:, 1] = bih1[H : 2 * H] + bhh1[H : 2 * H]
    bias[:, 2] = bhh0[2 * H :]
    bias[:, 3] = bhh1[2 * H :]
    bias[:, 4] = bih1[2 * H :]
    bias[:, 5] = c12
    bias[:, 6] = bfc
    bias[:, 7] = float(bout[0])

    v12 = np.concatenate([v1, v2], axis=1)  # [H, 2]

    shared = {
        "whhT0": np.ascontiguousarray(Whh0.T).astype(bf),
        "wihT0": wihT0_4x.astype(bf),
        "whhT1": np.ascontiguousarray(Whh1.T).astype(bf),
        "wihT1": np.ascontiguousarray(Wih1.T).astype(bf),
        "bias": bias,
        "v12": v12.astype(bf),
        "wfcT": np.ascontiguousarray(Wfc.T).astype(bf),
        "wout": np.ascontiguousarray(Wout[0].reshape(H, 1)).astype(bf),
        "identd": np.eye(H, dtype=f32).astype(bf),
        "identf": np.eye(H, dtype=f32),
        "ones1": np.ones((1, H), f32),
        "invde": inv_de.reshape(E, 1),
    }

    in_maps = []
    for ci in range(NC):
        n0 = ci * NL
        xc = x[n0 : n0 + NL, :T, :].astype(f32)  # [NL, T, DF]
        xa = np.ones((7, T, NL), f32)
        xa[:6] = xc.transpose(2, 1, 0)
        hn = NL // 2
        xA = xa[:, :, :hn].reshape(7, T * hn)
        xB = xa[:, :, hn:].reshape(7, T * hn)
        x4 = np.zeros((128, T * hn), f32)
        x4[0:7] = xA
        x4[32:39] = xA
        x4[64:71] = xB
        x4[96:103] = xB
        ghc = GH[n0 : n0 + NL].astype(f32)  # [NL, E]
        gh_nm = ghc.reshape(NCH, 128, E).transpose(1, 0, 2).reshape(128, NCH * E)
        invdv_nm = inv_dv[n0 : n0 + NL].reshape(NCH, 128).T.copy()
        m = dict(shared)
        m["x"] = x4.astype(bf)
        m["gh"] = np.ascontiguousarray(gh_nm)
        m["invdv"] = np.ascontiguousarray(invdv_nm)
        in_maps.append(m)
    return in_maps


def kernel(**inputs):
    if "nc" not in _CACHE:
        _CACHE["nc"] = _build_program()
    nc = _CACHE["nc"]
    in_maps = _prep_inputs(**inputs)
    res = run_bass_kernel_spmd(nc, in_maps, list(range(NC)))
    out = np.concatenate([res.results[i]["y"][0] for i in range(NC)])
    return out.astype(np.float32)


def _install_profile_shim():
    """Recreate the antenv.axon_hooks NTFF profile hook missing from this image."""
    import types
    import ctypes
    import contextlib

    if "antenv.axon_hooks" in sys.modules:
        return
    so_path = "/opt/axon/libaxon_pjrt.so"
    lib = ctypes.CDLL(so_path)
    lib.axon_start_nrt_profile.argtypes = [
        ctypes.POINTER(ctypes.c_int64), ctypes.c_size_t,
    ]
    lib.axon_start_nrt_profile.restype = ctypes.c_int64
    lib.axon_stop_nrt_profile.argtypes = [ctypes.c_char_p]
    lib.axon_stop_nrt_profile.restype = ctypes.c_int64

    @contextlib.contextmanager
    def _hook(output_dir, device_ids):
        import jax

        jax.devices()
        if device_ids:
            ids = (ctypes.c_int64 * len(device_ids))(*device_ids)
            rc = lib.axon_start_nrt_profile(ids, len(device_ids))
        else:
            rc = lib.axon_start_nrt_profile(None, 0)
        if rc != 0:
            raise RuntimeError(f"axon_start_nrt_profile rc={rc}")
        try:
            yield
        finally:
            n = lib.axon_stop_nrt_profile(str(output_dir).encode())
            print(f"profile: {n} file(s) written to {output_dir}")

    mod = types.ModuleType("antenv.axon_hooks")
    mod.get_axon_ntff_profile_hook = lambda: _hook
    mod.set_axon_ntff_profile_hook = lambda h: None
    sys.modules["antenv.axon_hooks"] = mod
    import antenv

    antenv.axon_hooks = mod

    import concourse.bass_utils as bu

    bu.upload_artifacts = lambda tmpdir: f"local://{tmpdir}"


def run_traced(inputs, tmpdir=None):
    """test.py helper: run with NTFF tracing, return (output, BassKernelResults)."""
    _install_profile_shim()
    if "nc" not in _CACHE:
        _CACHE["nc"] = _build_program()
    nc = _CACHE["nc"]
    in_maps = _prep_inputs(**inputs)
    res = run_bass_kernel_spmd(
        nc, in_maps, list(range(NC)), trace=True, tmpdir=tmpdir
    )
    out = np.concatenate([res.results[i]["y"][0] for i in range(NC)])
    return out.astype(np.float32), res


# revision 27
# speedup vs baseline: 1.3438x; 1.3438x over previous
"""HGAT model kernel for 8x Trainium2 NeuronCores.

Structure: 2-layer GRU (T=60, H=128) data-parallel over N=8192 nodes
(1024/core), software-pipelined so the tensor engine never idles long
enough to HAM-throttle; hypergraph attention collapsed through the E=30
hyperedge dim with a [E,1] vector AllReduce (s2) plus a [E,H] AllReduce.
"""

import sys

sys.path.insert(0, "/opt/trn_rl_repo")

import os
import numpy as np
import ml_dtypes

import concourse.bacc as bacc
import concourse.tile as tile
import concourse.mybir as mybir
from concourse.bass_utils import run_bass_kernel_spmd

F32 = mybir.dt.float32
BF16 = mybir.dt.bfloat16
AF = mybir.ActivationFunctionType
ALU = mybir.AluOpType
AX = mybir.AxisListType

N = 8192
T = int(os.environ.get("KERNEL_T", "60"))
DF = 6
H = 128
E = 30
NC = 8
NL = N // NC          # 1024 nodes per core
NCH = NL // 128       # 8 chunks of 128 nodes
SLOPE = 0.01
XQ = 4                # x streamed in 4 quarter-chunks
TQ = (T + XQ - 1) // XQ

_CACHE = {}

HALVES = (slice(0, 512), slice(512, 1024))


def _build_program():
    nc = bacc.Bacc("TRN2", target_bir_lowering=False, debug=False, num_devices=NC)

    dt = BF16

    # ---- DRAM I/O ----
    x_d = nc.dram_tensor("x", [128, T * (NL // 2)], dt, kind="ExternalInput")
    whhT0_d = nc.dram_tensor("whhT0", [H, 3 * H], dt, kind="ExternalInput")
    wihT0_d = nc.dram_tensor("wihT0", [128, 3 * H], dt, kind="ExternalInput")
    whhT1_d = nc.dram_tensor("whhT1", [H, 3 * H], dt, kind="ExternalInput")
    wihT1_d = nc.dram_tensor("wihT1", [H, 3 * H], dt, kind="ExternalInput")
    bias_d = nc.dram_tensor("bias", [H, 8], F32, kind="ExternalInput")
    v12_d = nc.dram_tensor("v12", [H, 2], dt, kind="ExternalInput")
    wfcT_d = nc.dram_tensor("wfcT", [H, H], dt, kind="ExternalInput")
    wout_d = nc.dram_tensor("wout", [H, 1], dt, kind="ExternalInput")
    identd_d = nc.dram_tensor("identd", [H, H], dt, kind="ExternalInput")
    identf_d = nc.dram_tensor("identf", [H, H], F32, kind="ExternalInput")
    gh_d = nc.dram_tensor("gh", [128, NCH * E], F32, kind="ExternalInput")
    invdv_d = nc.dram_tensor("invdv", [128, NCH], F32, kind="ExternalInput")
    invde_d = nc.dram_tensor("invde", [E, 1], F32, kind="ExternalInput")
    ones_d = nc.dram_tensor("ones1", [1, H], F32, kind="ExternalInput")
    y_d = nc.dram_tensor("y", [1, NL], F32, kind="ExternalOutput")

    with tile.TileContext(nc) as tc:
        with (
            tc.tile_pool(name="const", bufs=1) as cp,
            tc.tile_pool(name="xp", bufs=2) as xp,
            tc.tile_pool(name="hp", bufs=2) as hp,
            tc.tile_pool(name="wk", bufs=2) as wk,
            tc.tile_pool(name="pbs", bufs=1) as pbs,
            tc.tile_pool(name="dram", bufs=1, space="DRAM") as dp,
        ):
            # ---- load constants ----
            def cload(dram, shape, dtype):
                t_ = cp.tile(shape, dtype, tag=dram.name)
                nc.sync.dma_start(t_[:], dram[:])
                return t_

            whhT0 = cload(whhT0_d, [H, 3 * H], dt)
            wihT0 = cload(wihT0_d, [128, 3 * H], dt)
            whhT1 = cload(whhT1_d, [H, 3 * H], dt)
            wihT1 = cload(wihT1_d, [H, 3 * H], dt)
            bias = cload(bias_d, [H, 8], F32)
            v12 = cload(v12_d, [H, 2], dt)
            wfcT = cload(wfcT_d, [H, H], dt)
            wout = cload(wout_d, [H, 1], dt)
            identd = cload(identd_d, [H, H], dt)
            identf = cload(identf_d, [H, H], F32)
            gh = cload(gh_d, [128, NCH * E], F32)
            invdv = cload(invdv_d, [128, NCH], F32)
            invde = cload(invde_d, [E, 1], F32)
            ones1 = cload(ones_d, [1, H], F32)

            # ---- dummy AllReduce to warm the CC stream (overlaps early GRU) ----
            dum_in = dp.tile([1, 4], F32, tag="dum_in")
            dum_out = dp.tile([1, 4], F32, tag="dum_out")
            nc.sync.dma_start(dum_in[:], bias[0:1, 0:4])
            nc.gpsimd.collective_compute(
                "AllReduce", ALU.add,
                replica_groups=[list(range(NC))],
                ins=[dum_in.opt()], outs=[dum_out.opt()],
            )

            # gate column ranges in the 3H weight layout
            R, Z, G = slice(0, H), slice(H, 2 * H), slice(2 * H, 3 * H)

            HN = NL // 2  # 512 nodes per tower

            def make_tower(name, hf, po):
                h0z = hp.tile([H, HN], dt, tag="h0" + name)
                h1z = hp.tile([H, HN], dt, tag="h1" + name)
                nc.vector.memzero(h0z[:])
                nc.vector.memzero(h1z[:])
                return {"n": name, "hf": hf, "po": po, "h0": h0z, "h1": h1z, "l1": {}}

            twA = make_tower("a", HALVES[0], 0)
            twB = make_tower("b", HALVES[1], 64)
            st = {"xq": None}

            def xt_ap(t, po):
                # x slice for step t at row-tile partition offset po
                q = t // TQ
                off = (t % TQ) * HN
                return st["xq"][po : po + 7, off : off + HN]

            with tc.tile_pool(name="psA", bufs=1, space="PSUM") as psA:
                # per-tower psum: prz [128,1024] ([r|z], 2 banks), pr1/pz1 1 bank each

                def wt(tw, shape, tag):
                    return wk.tile(shape, dt, tag=tag + tw["n"], name=tag + tw["n"])

                def l1_mid(tw, t):
                    # wih1 parts (stop groups) + r/z sigmoids for L1 step t-1
                    l1 = tw["l1"]
                    pr1, pz1 = l1["pr1"], l1["pz1"]
                    nc.tensor.matmul(pr1[:], wihT1[:, R], tw["h0"][:],
                                     start=False, stop=True)
                    nc.tensor.matmul(pz1[:], wihT1[:, Z], tw["h0"][:],
                                     start=False, stop=True)
                    r1 = wt(tw, [H, HN], "r1")
                    nc.scalar.activation(r1[:], pr1[:], AF.Sigmoid, bias=bias[:, 0:1])
                    z1 = wt(tw, [H, HN], "z1")
                    nc.scalar.activation(z1[:], pz1[:], AF.Sigmoid, bias=bias[:, 1:2])
                    l1["r1"], l1["z1"] = r1, z1

                def l0_mm(tw, t):
                    prz = psA.tile([H, NL], F32, tag="prz" + tw["n"])
                    rz0 = wt(tw, [H, NL], "rz0")
                    po = tw["po"]
                    nc.tensor.matmul(prz[:, 0:HN], wihT0[po : po + 7, R],
                                     xt_ap(t, po), start=True, stop=False,
                                     tile_position=(po, 0))
                    nc.tensor.matmul(prz[:, HN:NL], wihT0[po + 32 : po + 39, Z],
                                     xt_ap(t, po + 32), start=True, stop=False,
                                     tile_position=(po + 32, 0))
                    nc.tensor.matmul(prz[:, 0:HN], whhT0[:, R], tw["h0"][:],
                                     start=False, stop=True)
                    nc.tensor.matmul(prz[:, HN:NL], whhT0[:, Z], tw["h0"][:],
                                     start=False, stop=True)
                    nc.scalar.activation(rz0[:], prz[:], AF.Sigmoid)
                    tw["prz"], tw["rz0"] = prz, rz0

                def l1_lateA(tw, t):
                    l1 = tw["l1"]
                    pr1, pz1 = l1["pr1"], l1["pz1"]
                    nc.tensor.matmul(pr1[:], whhT1[:, G], tw["h1"][:],
                                     start=True, stop=True)
                    nc.tensor.matmul(pz1[:], wihT1[:, G], tw["h0"][:],
                                     start=True, stop=False)
                    rh1 = wt(tw, [H, HN], "rh1")
                    nc.vector.scalar_tensor_tensor(
                        rh1[:], pr1[:], bias[:, 3:4], l1["r1"][:],
                        ALU.add, ALU.mult)
                    nc.tensor.matmul(pz1[:], identd[:], rh1[:],
                                     start=False, stop=True)

                def l0_stt(tw, t):
                    prz, rz0, po = tw["prz"], tw["rz0"], tw["po"]
                    nc.tensor.matmul(prz[:, 0:HN], whhT0[:, G], tw["h0"][:],
                                     start=True, stop=True)
                    nc.tensor.matmul(prz[:, HN:NL], wihT0[po : po + 7, G],
                                     xt_ap(t, po), start=True, stop=False,
                                     tile_position=(po, 0))
                    rh0 = wt(tw, [H, HN], "rh0")
                    nc.vector.scalar_tensor_tensor(
                        rh0[:], prz[:, 0:HN], bias[:, 2:3], rz0[:, 0:HN],
                        ALU.add, ALU.mult)
                    nc.tensor.matmul(prz[:, HN:NL], identd[:], rh0[:],
                                     start=False, stop=True)

                def l1_lateB(tw, t, ueng):
                    l1 = tw["l1"]
                    pz1 = l1["pz1"]
                    n1 = wt(tw, [H, HN], "n1")
                    nc.scalar.activation(n1[:], pz1[:], AF.Tanh, bias=bias[:, 4:5])
                    d1 = wt(tw, [H, HN], "d1")
                    nc.vector.tensor_sub(d1[:], tw["h1"][:], n1[:])
                    zd1 = wt(tw, [H, HN], "zd1")
                    nc.vector.tensor_mul(zd1[:], l1["z1"][:], d1[:])
                    h1n = hp.tile([H, HN], dt, tag="h1" + tw["n"])
                    nc.vector.tensor_add(h1n[:], n1[:], zd1[:])
                    tw["h1"] = h1n

                def l0_tail(tw, t, ueng):
                    prz, rz0 = tw["prz"], tw["rz0"]
                    n0 = wt(tw, [H, HN], "n0")
                    nc.scalar.activation(n0[:], prz[:, HN:NL], AF.Tanh)
                    d0 = wt(tw, [H, HN], "d0")
                    nc.vector.tensor_sub(d0[:], tw["h0"][:], n0[:])
                    zd0 = wt(tw, [H, HN], "zd0")
                    nc.vector.tensor_mul(zd0[:], rz0[:, HN:NL], d0[:])
                    h0n = hp.tile([H, HN], dt, tag="h0" + tw["n"])
                    nc.vector.tensor_add(h0n[:], n0[:], zd0[:])
                    tw["h0"] = h0n

                def l1_early(tw, t):
                    l1 = tw["l1"]
                    pr1 = psA.tile([H, HN], F32, tag="pr1" + tw["n"])
                    pz1 = psA.tile([H, HN], F32, tag="pz1" + tw["n"])
                    nc.tensor.matmul(pr1[:], whhT1[:, R], tw["h1"][:],
                                     start=True, stop=False)
                    nc.tensor.matmul(pz1[:], whhT1[:, Z], tw["h1"][:],
                                     start=True, stop=False)
                    l1["pr1"], l1["pz1"] = pr1, pz1

                def emit_step(tw, t):
                    ueng = nc.vector if t < 8 else nc.gpsimd
                    if t > 0:
                        l1_mid(tw, t)
                    l0_mm(tw, t)
                    if t > 0:
                        l1_lateA(tw, t)
                    l0_stt(tw, t)
                    if t > 0:
                        l1_lateB(tw, t, ueng)
                    l0_tail(tw, t, ueng)
                    l1_early(tw, t)

                def emit_l1_final(tw):
                    l1_mid(tw, T)
                    l1_lateA(tw, T)
                    l1_lateB(tw, T, nc.vector)

                for t in range(T + 1):
                    if t % TQ == 0 and t < T:
                        xq = xp.tile([128, TQ * HN], dt, tag="xq")
                        nc.sync.dma_start(xq[:], x_d[:, t * HN : (t + TQ) * HN])
                        st["xq"] = xq
                    if t < T:
                        emit_step(twA, t)
                    if t >= 1:
                        emit_step(twB, t - 1)
                emit_l1_final(twA)
                emit_l1_final(twB)

            h1ab = (twA["h1"], twB["h1"])  # two [H, 512] bf16 tiles

            # ================= attention tail =================
            hid_nm = pbs.tile([128, NL], F32)  # node-major hidden
            s12s = pbs.tile([128, 2 * NCH], F32)
            s2loc = pbs.tile([E, 1], F32)
            attd = pbs.tile([128, NCH * E], F32)
            attdT = pbs.tile([E, NL], F32)
            S_sb = pbs.tile([E, H], F32)

            with tc.tile_pool(name="psT", bufs=1, space="PSUM") as psT:
                ps_fc = psT.tile([128, NL], F32, tag="fc")      # 2 banks
                ps_tr = psT.tile([128, NL], dt, tag="tr")       # 1 bank
                ps_g2 = psT.tile([128, NL], F32, tag="g2")      # 2 banks
                ps_S = psT.tile([E, NL], F32, tag="S")          # 2 banks
                ps_ms = psT.tile([128, 512], F32, tag="misc")   # 1 bank

                def hid_chunk(c):
                    return h1ab[c // 4][:, 128 * (c % 4) : 128 * (c % 4 + 1)]

                # --- phase A: node-major transpose, s1/w2, local s2 ---
                for c in range(NCH):
                    cs = slice(128 * c, 128 * (c + 1))
                    nc.tensor.transpose(ps_tr[:, cs], hid_chunk(c), identd[:])
                nc.scalar.copy(hid_nm[:], ps_tr[:])
                for c in range(NCH):
                    nc.tensor.matmul(
                        ps_fc[:, 2 * c : 2 * c + 2], hid_chunk(c), v12[:],
                        start=True, stop=True,
                    )
                nc.scalar.copy(s12s[:], ps_fc[:, 0 : 2 * NCH])
                for c in range(NCH):
                    nc.tensor.matmul(
                        ps_fc[0:E, 31:32], gh[:, E * c : E * (c + 1)],
                        s12s[:, 2 * c + 1 : 2 * c + 2],
                        start=(c == 0), stop=(c == NCH - 1),
                    )
                nc.scalar.copy(s2loc[:], ps_fc[0:E, 31:32])

                s2_in = dp.tile([E, 1], F32, tag="s2_in")
                s2_out = dp.tile([E, 1], F32, tag="s2_out")
                nc.sync.dma_start(s2_in[:], s2loc[:])
                nc.gpsimd.collective_compute(
                    "AllReduce", ALU.add,
                    replica_groups=[list(range(NC))],
                    ins=[s2_in.opt()], outs=[s2_out.opt()],
                )
                s2g = pbs.tile([E, 1], F32)
                nc.sync.dma_start(s2g[:], s2_out[:])

                # --- phase B (overlaps AR#1): fc partial from hid ---
                for hi, hf in enumerate(HALVES):
                    nc.tensor.matmul(
                        ps_fc[:, hf], wfcT[:], h1ab[hi][:],
                        start=True, stop=False,
                    )

                # prewarm the exp table set while AR#1 is in flight
                exw = pbs.tile([1, 1], F32)
                nc.scalar.activation(exw[:], bias[0:1, 0:1], AF.Exp)

                # --- phase C: scores + softmax + local S ---
                s2c = pbs.tile([E, 1], F32)
                nc.vector.tensor_scalar_add(s2c[:], s2g[:], bias[0:E, 5:6])
                nc.tensor.transpose(ps_ms[0:1, 32:62], s2c[:], identf[0:E, 0:E])
                s2r = pbs.tile([1, E], F32)
                nc.scalar.copy(s2r[:], ps_ms[0:1, 32:62])
                nc.tensor.matmul(
                    ps_ms[:, 0:E], ones1[:], s2r[:], start=True, stop=True
                )
                s2b = pbs.tile([128, E], F32)
                nc.scalar.copy(s2b[:], ps_ms[:, 0:E])

                sc_t = pbs.tile([128, NCH * E], F32)
                lr_t = pbs.tile([128, NCH * E], F32)
                ex_t = pbs.tile([128, NCH * E], F32)
                nmx = pbs.tile([128, NCH], F32)
                se = pbs.tile([128, NCH], F32)
                rs = pbs.tile([128, NCH], F32)
                for c in range(NCH):
                    es = slice(E * c, E * (c + 1))
                    nc.vector.tensor_scalar_add(
                        sc_t[:, es], s2b[:], s12s[:, 2 * c : 2 * c + 1]
                    )
                for c in range(NCH):
                    es = slice(E * c, E * (c + 1))
                    nc.vector.scalar_tensor_tensor(
                        lr_t[:, es], sc_t[:, es], SLOPE, sc_t[:, es],
                        ALU.mult, ALU.max,
                    )
                for c in range(NCH):
                    es = slice(E * c, E * (c + 1))
                    nc.vector.tensor_reduce(
                        nmx[:, c : c + 1], lr_t[:, es], AX.X, ALU.max, negate=True
                    )
                for c in range(NCH):
                    es = slice(E * c, E * (c + 1))
                    nc.scalar.activation(
                        ex_t[:, es], lr_t[:, es], AF.Exp,
                        bias=nmx[:, c : c + 1],
                    )
                for c in range(NCH):
                    es = slice(E * c, E * (c + 1))
                    nc.vector.tensor_reduce(
                        se[:, c : c + 1], ex_t[:, es], AX.X, ALU.add
                    )
                nc.vector.reciprocal(rs[:], se[:])
                for c in range(NCH):
                    es = slice(E * c, E * (c + 1))
                    nc.vector.tensor_scalar(
                        attd[:, es], ex_t[:, es], rs[:, c : c + 1],
                        invdv[:, c : c + 1], ALU.mult, ALU.mult,
                    )
                for c in range(NCH):
                    cs = slice(128 * c, 128 * (c + 1))
                    es = slice(E * c, E * (c + 1))
                    nc.tensor.matmul(
                        ps_S[:, 0:H], attd[:, es], hid_nm[:, cs],
                        start=(c == 0), stop=(c == NCH - 1),
                    )
                nc.scalar.copy(S_sb[:], ps_S[:, 0:H])

                S_in = dp.tile([E, H], F32, tag="S_in")
                S_out = dp.tile([E, H], F32, tag="S_out")
                nc.sync.dma_start(S_in[:], S_sb[:])
                nc.gpsimd.collective_compute(
                    "AllReduce", ALU.add,
                    replica_groups=[list(range(NC))],
                    ins=[S_in.opt()], outs=[S_out.opt()],
                )
                SF = pbs.tile([E, H], F32)
                nc.sync.dma_start(SF[:], S_out[:])

                # --- phase D (overlaps AR#2): attd transposes ---
                for c in range(NCH):
                    nc.tensor.transpose(
                        ps_S[:, 128 * c : 128 * (c + 1)],
                        attd[:, E * c : E * (c + 1)], identf[:],
                    )
                nc.scalar.copy(attdT[:], ps_S[:])

                # --- phase E: g2, fc, out ---
                Sd = pbs.tile([E, H], F32)
                nc.vector.tensor_scalar_mul(Sd[:], SF[:], invde[:])
                for hf in HALVES:
                    nc.tensor.matmul(
                        ps_g2[:, hf], Sd[:], attdT[:, hf],
                        start=True, stop=True,
                    )
                g2s = pbs.tile([128, NL], dt)
                nc.scalar.copy(g2s[:], ps_g2[:])
                for hf in HALVES:
                    nc.tensor.matmul(
                        ps_fc[:, hf], wfcT[:], g2s[:, hf],
                        start=False, stop=True,
                    )
                fcb = pbs.tile([128, NL], F32)
                nc.scalar.activation(fcb[:], ps_fc[:], AF.Identity, bias=bias[:, 6:7])
                fc2 = pbs.tile([128, NL], dt)
                for hf in HALVES:
                    nc.vector.scalar_tensor_tensor(
                        fc2[:, hf], fcb[:, hf], SLOPE, fcb[:, hf],
                        ALU.mult, ALU.max,
                    )
                y_sb = pbs.tile([1, NL], F32)
                for hf in HALVES:
                    nc.tensor.matmul(
                        ps_ms[0:1, 0:512], wout[:], fc2[:, hf],
                        start=True, stop=True,
                    )
                    nc.vector.tensor_scalar_add(
                        y_sb[0:1, hf], ps_ms[0:1, 0:512], bias[0:1, 7:8]
                    )
                nc.sync.dma_start(y_d[:], y_sb[:])

    nc.finalize()
    return nc


def _prep_inputs(x, GH, Wih0, Whh0, bih0, bhh0, Wih1, Whh1, bih1, bhh1,
                 Wt, bt, a, Wfc, bfc, Wout, bout):
    bf = ml_dtypes.bfloat16
    f32 = np.float32

    a1, a2 = a[:H, 0].astype(f32), a[H:, 0].astype(f32)
    v1 = (Wt.T.astype(f32) @ a1).reshape(H, 1)
    v2 = (Wt.T.astype(f32) @ a2).reshape(H, 1)
    c12 = float(bt.astype(f32) @ a1 + bt.astype(f32) @ a2)

    de = GH.astype(f32).sum(axis=0)
    dv = GH.astype(f32).sum(axis=1) / 2.0
    inv_de = np.where(de != 0, 1.0 / np.where(de != 0, de, 1.0), 0.0).astype(f32)
    inv_dv = np.where(dv != 0, 1.0 / np.where(dv != 0, dv, 1.0), 0.0).astype(f32)

    wihT0_aug = np.zeros((7, 3 * H), f32)
    wihT0_aug[:6] = Wih0.T
    wihT0_aug[6, 0:H] = bih0[0:H] + bhh0[0:H]
    wihT0_aug[6, H : 2 * H] = bih0[H : 2 * H] + bhh0[H : 2 * H]
    wihT0_aug[6, 2 * H :] = bih0[2 * H :]
    wihT0_4x = np.zeros((128, 3 * H), f32)
    for po in (0, 32, 64, 96):
        wihT0_4x[po : po + 7] = wihT0_aug

    bias = np.zeros((H, 8), f32)
    bias[:, 0] = bih1[0:H] + bhh1[0:H]
    bias[:, 1] = bih1[H : 2 * H] + bhh1[H : 2 * H]
    bias[:, 2] = bhh0[2 * H :]
    bias[:, 3] = bhh1[2 * H :]
    bias[:, 4] = bih1[2 * H :]
    bias[:, 5] = c12
    bias[:, 6] = bfc
    bias[:, 7] = float(bout[0])

    v12 = np.concatenate([v1, v2], axis=1)  # [H, 2]

    shared = {
        "whhT0": np.ascontiguousarray(Whh0.T).astype(bf),
        "wihT0": wihT0_4x.astype(bf),
        "whhT1": np.ascontiguousarray(Whh1.T).astype(bf),
        "wihT1": np.ascontiguousarray(Wih1.T).astype(bf),
        "bias": bias,
        "v12": v12.astype(bf),
        "wfcT": np.ascontiguousarray(Wfc.T).astype(bf),
        "wout": np.ascontiguousarray(Wout[0].reshape(H, 1)).astype(bf),
        "identd": np.eye(H, dtype=f32).astype(bf),
        "identf": np.eye(H, dtype=f32),
        "ones1": np.ones((1, H), f32),
        "invde": inv_de.reshape(E, 1),
    }

    in_maps = []
    for ci in range(NC):
        n0 = ci * NL
        xc = x[n0 : n0 + NL, :T, :].astype(f32)  # [NL, T, DF]
        xa = np.ones((7, T, NL), f32)
        xa[:6] = xc.transpose(2, 1, 0)
        hn = NL // 2
        xA = xa[:, :, :hn].reshape(7, T * hn)
        xB = xa[:, :, hn:].reshape(7, T * hn)
        x4 = np.zeros((128, T * hn), f32)
        x4[0:7] = xA
        x4[32:39] = xA
        x4[64:71] = xB
        x4[96:103] = xB
        ghc = GH[n0 : n0 + NL].astype(f32)  # [NL, E]
        gh_nm = ghc.reshape(NCH, 128, E).transpose(1, 0, 2).reshape(128, NCH * E)
        invdv_nm = inv_dv[n0 : n0 + NL].reshape(NCH, 128).T.copy()
        m = dict(shared)
        m["x"] = x4.astype(bf)
        m["gh"] = np.ascontiguousarray(gh_nm)
        m["invdv"] = np.ascontiguousarray(invdv_nm)
        in_maps.append(m)
    return in_maps


def kernel(**inputs):
    if "nc" not in _CACHE:
        _CACHE["nc"] = _build_program()
    nc = _CACHE["nc"]
    in_maps = _prep_inputs(**inputs)
    res = run_bass_kernel_spmd(nc, in_maps, list(range(NC)))
    out = np.concatenate([res.results[i]["y"][0] for i in range(NC)])
    return out.astype(np.float32)


def _install_profile_shim():
    """Recreate the antenv.axon_hooks NTFF profile hook missing from this image."""
    import types
    import ctypes
    import contextlib

    if "antenv.axon_hooks" in sys.modules:
        return
    so_path = "/opt/axon/libaxon_pjrt.so"
    lib = ctypes.CDLL(so_path)
    lib.axon_start_nrt_profile.argtypes = [
        ctypes.POINTER(ctypes.c_int64), ctypes.c_size_t,
    ]
    lib.axon_start_nrt_profile.restype = ctypes.c_int64
    lib.axon_stop_nrt_profile.argtypes = [ctypes.c_char_p]
    lib.axon_stop_nrt_profile.restype = ctypes.c_int64

    @contextlib.contextmanager
    def _hook(output_dir, device_ids):
        import jax

        jax.devices()
        if device_ids:
            ids = (ctypes.c_int64 * len(device_ids))(*device_ids)
            rc = lib.axon_start_nrt_profile(ids, len(device_ids))
        else:
            rc = lib.axon_start_nrt_profile(None, 0)
        if rc != 0:
            raise RuntimeError(f"axon_start_nrt_profile rc={rc}")
        try:
            yield
        finally:
            n = lib.axon_stop_nrt_profile(str(output_dir).encode())
            print(f"profile: {n} file(s) written to {output_dir}")

    mod = types.ModuleType("antenv.axon_hooks")
    mod.get_axon_ntff_profile_hook = lambda: _hook
    mod.set_axon_ntff_profile_hook = lambda h: None
    sys.modules["antenv.axon_hooks"] = mod
    import antenv

    antenv.axon_hooks = mod

    import concourse.bass_utils as bu

    bu.upload_artifacts = lambda tmpdir: f"local://{tmpdir}"


def run_traced(inputs, tmpdir=None):
    """test.py helper: run with NTFF tracing, return (output, BassKernelResults)."""
    _install_profile_shim()
    if "nc" not in _CACHE:
        _CACHE["nc"] = _build_program()
    nc = _CACHE["nc"]
    in_maps = _prep_inputs(**inputs)
    res = run_bass_kernel_spmd(
        nc, in_maps, list(range(NC)), trace=True, tmpdir=tmpdir
    )
    out = np.concatenate([res.results[i]["y"][0] for i in range(NC)])
    return out.astype(np.float32), res


# revision 28
# speedup vs baseline: 1.3553x; 1.0085x over previous
"""HGAT model kernel for 8x Trainium2 NeuronCores.

Structure: 2-layer GRU (T=60, H=128) data-parallel over N=8192 nodes
(1024/core), software-pipelined so the tensor engine never idles long
enough to HAM-throttle; hypergraph attention collapsed through the E=30
hyperedge dim with a [E,1] vector AllReduce (s2) plus a [E,H] AllReduce.
"""

import sys

sys.path.insert(0, "/opt/trn_rl_repo")

import os
import numpy as np
import ml_dtypes

import concourse.bacc as bacc
import concourse.tile as tile
import concourse.mybir as mybir
from concourse.bass_utils import run_bass_kernel_spmd

F32 = mybir.dt.float32
BF16 = mybir.dt.bfloat16
AF = mybir.ActivationFunctionType
ALU = mybir.AluOpType
AX = mybir.AxisListType

N = 8192
T = int(os.environ.get("KERNEL_T", "60"))
DF = 6
H = 128
E = 30
NC = 8
NL = N // NC          # 1024 nodes per core
NCH = NL // 128       # 8 chunks of 128 nodes
SLOPE = 0.01
XQ = 4                # x streamed in 4 quarter-chunks
TQ = (T + XQ - 1) // XQ

_CACHE = {}

HALVES = (slice(0, 512), slice(512, 1024))


def _build_program():
    nc = bacc.Bacc("TRN2", target_bir_lowering=False, debug=False, num_devices=NC)

    dt = BF16

    # ---- DRAM I/O ----
    x_d = nc.dram_tensor("x", [128, T * (NL // 2)], dt, kind="ExternalInput")
    whhT0_d = nc.dram_tensor("whhT0", [H, 3 * H], dt, kind="ExternalInput")
    wihT0_d = nc.dram_tensor("wihT0", [128, 3 * H], dt, kind="ExternalInput")
    whhT1_d = nc.dram_tensor("whhT1", [H, 3 * H], dt, kind="ExternalInput")
    wihT1_d = nc.dram_tensor("wihT1", [H, 3 * H], dt, kind="ExternalInput")
    bias_d = nc.dram_tensor("bias", [H, 8], F32, kind="ExternalInput")
    v12_d = nc.dram_tensor("v12", [H, 2], dt, kind="ExternalInput")
    wfcT_d = nc.dram_tensor("wfcT", [H, H], dt, kind="ExternalInput")
    wout_d = nc.dram_tensor("wout", [H, 1], dt, kind="ExternalInput")
    identd_d = nc.dram_tensor("identd", [H, H], dt, kind="ExternalInput")
    identf_d = nc.dram_tensor("identf", [H, H], F32, kind="ExternalInput")
    gh_d = nc.dram_tensor("gh", [128, NCH * E], F32, kind="ExternalInput")
    invdv_d = nc.dram_tensor("invdv", [128, NCH], F32, kind="ExternalInput")
    invde_d = nc.dram_tensor("invde", [E, 1], F32, kind="ExternalInput")
    ones_d = nc.dram_tensor("ones1", [1, H], F32, kind="ExternalInput")
    y_d = nc.dram_tensor("y", [1, NL], F32, kind="ExternalOutput")

    with tile.TileContext(nc) as tc:
        with (
            tc.tile_pool(name="const", bufs=1) as cp,
            tc.tile_pool(name="xp", bufs=2) as xp,
            tc.tile_pool(name="hp", bufs=2) as hp,
            tc.tile_pool(name="wk", bufs=2) as wk,
            tc.tile_pool(name="pbs", bufs=1) as pbs,
            tc.tile_pool(name="dram", bufs=1, space="DRAM") as dp,
        ):
            # ---- load constants ----
            def cload(dram, shape, dtype):
                t_ = cp.tile(shape, dtype, tag=dram.name)
                nc.sync.dma_start(t_[:], dram[:])
                return t_

            whhT0 = cload(whhT0_d, [H, 3 * H], dt)
            wihT0 = cload(wihT0_d, [128, 3 * H], dt)
            whhT1 = cload(whhT1_d, [H, 3 * H], dt)
            wihT1 = cload(wihT1_d, [H, 3 * H], dt)
            bias = cload(bias_d, [H, 8], F32)
            v12 = cload(v12_d, [H, 2], dt)
            wfcT = cload(wfcT_d, [H, H], dt)
            wout = cload(wout_d, [H, 1], dt)
            identd = cload(identd_d, [H, H], dt)
            identf = cload(identf_d, [H, H], F32)
            gh = cload(gh_d, [128, NCH * E], F32)
            invdv = cload(invdv_d, [128, NCH], F32)
            invde = cload(invde_d, [E, 1], F32)
            ones1 = cload(ones_d, [1, H], F32)

            # ---- dummy AllReduce to warm the CC stream (overlaps early GRU) ----
            dum_in = dp.tile([1, 4], F32, tag="dum_in")
            dum_out = dp.tile([1, 4], F32, tag="dum_out")
            nc.sync.dma_start(dum_in[:], bias[0:1, 0:4])
            nc.gpsimd.collective_compute(
                "AllReduce", ALU.add,
                replica_groups=[list(range(NC))],
                ins=[dum_in.opt()], outs=[dum_out.opt()],
            )

            # gate column ranges in the 3H weight layout
            R, Z, G = slice(0, H), slice(H, 2 * H), slice(2 * H, 3 * H)

            HN = NL // 2  # 512 nodes per tower

            def make_tower(name, hf, po):
                h0z = hp.tile([H, HN], dt, tag="h0" + name)
                h1z = hp.tile([H, HN], dt, tag="h1" + name)
                nc.vector.memzero(h0z[:])
                nc.vector.memzero(h1z[:])
                return {"n": name, "hf": hf, "po": po, "h0": h0z, "h1": h1z, "l1": {}}

            twA = make_tower("a", HALVES[0], 0)
            twB = make_tower("b", HALVES[1], 64)
            st = {"xq": None}

            def xt_ap(t, po):
                # x slice for step t at row-tile partition offset po
                q = t // TQ
                off = (t % TQ) * HN
                return st["xq"][po : po + 7, off : off + HN]

            with tc.tile_pool(name="psA", bufs=1, space="PSUM") as psA:
                # per-tower psum: prz [128,1024] ([r|z], 2 banks), pr1/pz1 1 bank each

                def wt(tw, shape, tag):
                    return wk.tile(shape, dt, tag=tag + tw["n"], name=tag + tw["n"])

                def l1_mid(tw, t):
                    # wih1 parts (stop groups) + r/z sigmoids for L1 step t-1
                    l1 = tw["l1"]
                    pr1, pz1 = l1["pr1"], l1["pz1"]
                    nc.tensor.matmul(pr1[:], wihT1[:, R], tw["h0"][:],
                                     start=False, stop=True)
                    nc.tensor.matmul(pz1[:], wihT1[:, Z], tw["h0"][:],
                                     start=False, stop=True)
                    r1 = wt(tw, [H, HN], "r1")
                    nc.scalar.activation(r1[:], pr1[:], AF.Sigmoid, bias=bias[:, 0:1])
                    z1 = wt(tw, [H, HN], "z1")
                    nc.scalar.activation(z1[:], pz1[:], AF.Sigmoid, bias=bias[:, 1:2])
                    l1["r1"], l1["z1"] = r1, z1

                def l0_mm(tw, t):
                    prz = psA.tile([H, NL], F32, tag="prz" + tw["n"])
                    rz0 = wt(tw, [H, NL], "rz0")
                    po = tw["po"]
                    nc.tensor.matmul(prz[:, 0:HN], wihT0[po : po + 7, R],
                                     xt_ap(t, po), start=True, stop=False,
                                     tile_position=(po, 0))
                    nc.tensor.matmul(prz[:, HN:NL], wihT0[po + 32 : po + 39, Z],
                                     xt_ap(t, po + 32), start=True, stop=False,
                                     tile_position=(po + 32, 0))
                    nc.tensor.matmul(prz[:, 0:HN], whhT0[:, R], tw["h0"][:],
                                     start=False, stop=True)
                    nc.tensor.matmul(prz[:, HN:NL], whhT0[:, Z], tw["h0"][:],
                                     start=False, stop=True)
                    nc.scalar.activation(rz0[:], prz[:], AF.Sigmoid)
                    tw["prz"], tw["rz0"] = prz, rz0

                def l1_lateA(tw, t):
                    l1 = tw["l1"]
                    pr1, pz1 = l1["pr1"], l1["pz1"]
                    nc.tensor.matmul(pr1[:], whhT1[:, G], tw["h1"][:],
                                     start=True, stop=True)
                    nc.tensor.matmul(pz1[:], wihT1[:, G], tw["h0"][:],
                                     start=True, stop=False)
                    rh1 = wt(tw, [H, HN], "rh1")
                    nc.vector.scalar_tensor_tensor(
                        rh1[:], pr1[:], bias[:, 3:4], l1["r1"][:],
                        ALU.add, ALU.mult)
                    nc.tensor.matmul(pz1[:], identd[:], rh1[:],
                                     start=False, stop=True)

                def l0_stt(tw, t):
                    prz, rz0, po = tw["prz"], tw["rz0"], tw["po"]
                    nc.tensor.matmul(prz[:, 0:HN], whhT0[:, G], tw["h0"][:],
                                     start=True, stop=True)
                    nc.tensor.matmul(prz[:, HN:NL], wihT0[po : po + 7, G],
                                     xt_ap(t, po), start=True, stop=False,
                                     tile_position=(po, 0))
                    rh0 = wt(tw, [H, HN], "rh0")
                    nc.vector.scalar_tensor_tensor(
                        rh0[:], prz[:, 0:HN], bias[:, 2:3], rz0[:, 0:HN],
                        ALU.add, ALU.mult)
                    nc.tensor.matmul(prz[:, HN:NL], identd[:], rh0[:],
                                     start=False, stop=True)

                def l1_lateB(tw, t, ueng):
                    l1 = tw["l1"]
                    pz1 = l1["pz1"]
                    n1 = wt(tw, [H, HN], "n1")
                    nc.scalar.activation(n1[:], pz1[:], AF.Tanh, bias=bias[:, 4:5])
                    d1 = wt(tw, [H, HN], "d1")
                    nc.vector.tensor_sub(d1[:], tw["h1"][:], n1[:])
                    zd1 = wt(tw, [H, HN], "zd1")
                    nc.vector.tensor_mul(zd1[:], l1["z1"][:], d1[:])
                    h1n = hp.tile([H, HN], dt, tag="h1" + tw["n"])
                    nc.vector.tensor_add(h1n[:], n1[:], zd1[:])
                    tw["h1"] = h1n

                def l0_tail(tw, t, ueng):
                    prz, rz0 = tw["prz"], tw["rz0"]
                    n0 = wt(tw, [H, HN], "n0")
                    nc.scalar.activation(n0[:], prz[:, HN:NL], AF.Tanh)
                    d0 = wt(tw, [H, HN], "d0")
                    nc.vector.tensor_sub(d0[:], tw["h0"][:], n0[:])
                    zd0 = wt(tw, [H, HN], "zd0")
                    nc.vector.tensor_mul(zd0[:], rz0[:, HN:NL], d0[:])
                    h0n = hp.tile([H, HN], dt, tag="h0" + tw["n"])
                    nc.vector.tensor_add(h0n[:], n0[:], zd0[:])
                    tw["h0"] = h0n

                def l1_early(tw, t):
                    l1 = tw["l1"]
                    pr1 = psA.tile([H, HN], F32, tag="pr1" + tw["n"])
                    pz1 = psA.tile([H, HN], F32, tag="pz1" + tw["n"])
                    nc.tensor.matmul(pr1[:], whhT1[:, R], tw["h1"][:],
                                     start=True, stop=False)
                    nc.tensor.matmul(pz1[:], whhT1[:, Z], tw["h1"][:],
                                     start=True, stop=False)
                    l1["pr1"], l1["pz1"] = pr1, pz1

                def emit_step(tw, t):
                    ueng = nc.vector if t < 8 else nc.gpsimd
                    if t > 0:
                        l1_mid(tw, t)
                    l0_mm(tw, t)
                    if t > 0:
                        l1_lateA(tw, t)
                    l0_stt(tw, t)
                    if t > 0:
                        l1_lateB(tw, t, ueng)
                    l0_tail(tw, t, ueng)
                    l1_early(tw, t)

                def emit_l1_final(tw):
                    l1_mid(tw, T)
                    l1_lateA(tw, T)
                    l1_lateB(tw, T, nc.vector)

                for t in range(T + 1):
                    if t % TQ == 0 and t < T:
                        xq = xp.tile([128, TQ * HN], dt, tag="xq")
                        nc.sync.dma_start(xq[:], x_d[:, t * HN : (t + TQ) * HN])
                        st["xq"] = xq
                    if t < T:
                        emit_step(twA, t)
                    if t >= 1:
                        emit_step(twB, t - 1)
                emit_l1_final(twA)
                emit_l1_final(twB)

            h1ab = (twA["h1"], twB["h1"])  # two [H, 512] bf16 tiles

            # ================= attention tail =================
            hid_nm = pbs.tile([128, NL], F32)  # node-major hidden
            s12s = pbs.tile([128, 2 * NCH], F32)
            s2loc = pbs.tile([E, 1], F32)
            attd = pbs.tile([128, NCH * E], F32)
            attdT = pbs.tile([E, NL], F32)
            S_sb = pbs.tile([E, H], F32)

            with tc.tile_pool(name="psT", bufs=1, space="PSUM") as psT:
                ps_fc = psT.tile([128, NL], F32, tag="fc")      # 2 banks
                ps_tr = psT.tile([128, NL], dt, tag="tr")       # 1 bank
                ps_g2 = psT.tile([128, NL], F32, tag="g2")      # 2 banks
                ps_S = psT.tile([E, NL], F32, tag="S")          # 2 banks
                ps_ms = psT.tile([128, 512], F32, tag="misc")   # 1 bank

                def hid_chunk(c):
                    return h1ab[c // 4][:, 128 * (c % 4) : 128 * (c % 4 + 1)]

                # --- phase A: node-major transpose, s1/w2, local s2 ---
                for c in range(NCH):
                    cs = slice(128 * c, 128 * (c + 1))
                    nc.tensor.transpose(ps_tr[:, cs], hid_chunk(c), identd[:])
                nc.scalar.copy(hid_nm[:], ps_tr[:])
                for c in range(NCH):
                    nc.tensor.matmul(
                        ps_fc[:, 2 * c : 2 * c + 2], hid_chunk(c), v12[:],
                        start=True, stop=True,
                    )
                nc.scalar.copy(s12s[:], ps_fc[:, 0 : 2 * NCH])
                for c in range(NCH):
                    nc.tensor.matmul(
                        ps_fc[0:E, 31:32], gh[:, E * c : E * (c + 1)],
                        s12s[:, 2 * c + 1 : 2 * c + 2],
                        start=(c == 0), stop=(c == NCH - 1),
                    )
                nc.scalar.copy(s2loc[:], ps_fc[0:E, 31:32])

                s2_in = dp.tile([E, 1], F32, tag="s2_in")
                s2_out = dp.tile([E, 1], F32, tag="s2_out")
                nc.sync.dma_start(s2_in[:], s2loc[:])
                nc.gpsimd.collective_compute(
                    "AllReduce", ALU.add,
                    replica_groups=[list(range(NC))],
                    ins=[s2_in.opt()], outs=[s2_out.opt()],
                )
                s2g = pbs.tile([E, 1], F32)
                nc.sync.dma_start(s2g[:], s2_out[:])

                # --- phase B (overlaps AR#1): fc partial from hid ---
                for hi, hf in enumerate(HALVES):
                    nc.tensor.matmul(
                        ps_fc[:, hf], wfcT[:], h1ab[hi][:],
                        start=True, stop=False,
                    )

                # prewarm the exp table set while AR#1 is in flight
                exw = pbs.tile([1, 1], F32)
                nc.scalar.activation(exw[:], bias[0:1, 0:1], AF.Exp)

                # --- phase C: scores + softmax + local S ---
                s2c = pbs.tile([E, 1], F32)
                nc.vector.tensor_scalar_add(s2c[:], s2g[:], bias[0:E, 5:6])
                nc.tensor.transpose(ps_ms[0:1, 32:62], s2c[:], identf[0:E, 0:E])
                s2r = pbs.tile([1, E], F32)
                nc.scalar.copy(s2r[:], ps_ms[0:1, 32:62])
                nc.tensor.matmul(
                    ps_ms[:, 0:E], ones1[:], s2r[:], start=True, stop=True
                )
                s2b = pbs.tile([128, E], F32)
                nc.scalar.copy(s2b[:], ps_ms[:, 0:E])

                sc_t = pbs.tile([128, NCH * E], F32)
                lr_t = pbs.tile([128, NCH * E], F32)
                ex_t = pbs.tile([128, NCH * E], F32)
                nmx = pbs.tile([128, NCH], F32)
                se = pbs.tile([128, NCH], F32)
                rs = pbs.tile([128, NCH], F32)
                for c in range(NCH):
                    es = slice(E * c, E * (c + 1))
                    nc.vector.tensor_scalar_add(
                        sc_t[:, es], s2b[:], s12s[:, 2 * c : 2 * c + 1]
                    )
                for c in range(NCH):
                    es = slice(E * c, E * (c + 1))
                    nc.vector.scalar_tensor_tensor(
                        lr_t[:, es], sc_t[:, es], SLOPE, sc_t[:, es],
                        ALU.mult, ALU.max,
                    )
                for c in range(NCH):
                    es = slice(E * c, E * (c + 1))
                    nc.vector.tensor_reduce(
                        nmx[:, c : c + 1], lr_t[:, es], AX.X, ALU.max, negate=True
                    )
                for c in range(NCH):
                    es = slice(E * c, E * (c + 1))
                    nc.scalar.activation(
                        ex_t[:, es], lr_t[:, es], AF.Exp,
                        bias=nmx[:, c : c + 1],
                    )
                for c in range(NCH):
                    es = slice(E * c, E * (c + 1))
                    nc.vector.tensor_reduce(
                        se[:, c : c + 1], ex_t[:, es], AX.X, ALU.add
                    )
                nc.vector.reciprocal(rs[:], se[:])
                for c in range(NCH):
                    es = slice(E * c, E * (c + 1))
                    nc.vector.tensor_scalar(
                        attd[:, es], ex_t[:, es], rs[:, c : c + 1],
                        invdv[:, c : c + 1], ALU.mult, ALU.mult,
                    )
                for c in range(NCH):
                    cs = slice(128 * c, 128 * (c + 1))
                    es = slice(E * c, E * (c + 1))
                    nc.tensor.matmul(
                        ps_S[:, 0:H], attd[:, es], hid_nm[:, cs],
                        start=(c == 0), stop=(c == NCH - 1),
                    )
                nc.scalar.copy(S_sb[:], ps_S[:, 0:H])

                S_in = dp.tile([E, H], F32, tag="S_in")
                S_out = dp.tile([E, H], F32, tag="S_out")
                nc.sync.dma_start(S_in[:], S_sb[:])
                nc.gpsimd.collective_compute(
                    "AllReduce", ALU.add,
                    replica_groups=[list(range(NC))],
                    ins=[S_in.opt()], outs=[S_out.opt()],
                )
                SF = pbs.tile([E, H], F32)
                nc.sync.dma_start(SF[:], S_out[:])

                # --- phase D (overlaps AR#2): attd transposes ---
                for c in range(NCH):
                    nc.tensor.transpose(
                        ps_S[:, 128 * c : 128 * (c + 1)],
                        attd[:, E * c : E * (c + 1)], identf[:],
                    )
                nc.scalar.copy(attdT[:], ps_S[:])

                # --- phase E: g2, fc, out (pipelined by halves) ---
                Sd = pbs.tile([E, H], F32)
                nc.vector.tensor_scalar_mul(Sd[:], SF[:], invde[:])
                g2s = pbs.tile([128, NL], dt)
                fcb = pbs.tile([128, NL], F32)
                fc2 = pbs.tile([128, NL], dt)
                y_sb = pbs.tile([1, NL], F32)
                for hf in HALVES:
                    nc.tensor.matmul(
                        ps_g2[:, hf], Sd[:], attdT[:, hf],
                        start=True, stop=True,
                    )
                    nc.scalar.copy(g2s[:, hf], ps_g2[:, hf])
                    nc.tensor.matmul(
                        ps_fc[:, hf], wfcT[:], g2s[:, hf],
                        start=False, stop=True,
                    )
                    nc.scalar.activation(
                        fcb[:, hf], ps_fc[:, hf], AF.Identity, bias=bias[:, 6:7]
                    )
                    nc.vector.scalar_tensor_tensor(
                        fc2[:, hf], fcb[:, hf], SLOPE, fcb[:, hf],
                        ALU.mult, ALU.max,
                    )
                    nc.tensor.matmul(
                        ps_ms[0:1, 0:512], wout[:], fc2[:, hf],
                        start=True, stop=True,
                    )
                    nc.vector.tensor_scalar_add(
                        y_sb[0:1, hf], ps_ms[0:1, 0:512], bias[0:1, 7:8]
                    )
                nc.sync.dma_start(y_d[:], y_sb[:])

    nc.finalize()
    return nc


def _prep_inputs(x, GH, Wih0, Whh0, bih0, bhh0, Wih1, Whh1, bih1, bhh1,
                 Wt, bt, a, Wfc, bfc, Wout, bout):
    bf = ml_dtypes.bfloat16
    f32 = np.float32

    a1, a2 = a[:H, 0].astype(f32), a[H:, 0].astype(f32)
    v1 = (Wt.T.astype(f32) @ a1).reshape(H, 1)
    v2 = (Wt.T.astype(f32) @ a2).reshape(H, 1)
    c12 = float(bt.astype(f32) @ a1 + bt.astype(f32) @ a2)

    de = GH.astype(f32).sum(axis=0)
    dv = GH.astype(f32).sum(axis=1) / 2.0
    inv_de = np.where(de != 0, 1.0 / np.where(de != 0, de, 1.0), 0.0).astype(f32)
    inv_dv = np.where(dv != 0, 1.0 / np.where(dv != 0, dv, 1.0), 0.0).astype(f32)

    wihT0_aug = np.zeros((7, 3 * H), f32)
    wihT0_aug[:6] = Wih0.T
    wihT0_aug[6, 0:H] = bih0[0:H] + bhh0[0:H]
    wihT0_aug[6, H : 2 * H] = bih0[H : 2 * H] + bhh0[H : 2 * H]
    wihT0_aug[6, 2 * H :] = bih0[2 * H :]
    wihT0_4x = np.zeros((128, 3 * H), f32)
    for po in (0, 32, 64, 96):
        wihT0_4x[po : po + 7] = wihT0_aug

    bias = np.zeros((H, 8), f32)
    bias[:, 0] = bih1[0:H] + bhh1[0:H]
    bias[:, 1] = bih1[H : 2 * H] + bhh1[H : 2 * H]
    bias[:, 2] = bhh0[2 * H :]
    bias[:, 3] = bhh1[2 * H :]
    bias[:, 4] = bih1[2 * H :]
    bias[:, 5] = c12
    bias[:, 6] = bfc
    bias[:, 7] = float(bout[0])

    v12 = np.concatenate([v1, v2], axis=1)  # [H, 2]

    shared = {
        "whhT0": np.ascontiguousarray(Whh0.T).astype(bf),
        "wihT0": wihT0_4x.astype(bf),
        "whhT1": np.ascontiguousarray(Whh1.T).astype(bf),
        "wihT1": np.ascontiguousarray(Wih1.T).astype(bf),
        "bias": bias,
        "v12": v12.astype(bf),
        "wfcT": np.ascontiguousarray(Wfc.T).astype(bf),
        "wout": np.ascontiguousarray(Wout[0].reshape(H, 1)).astype(bf),
        "identd": np.eye(H, dtype=f32).astype(bf),
        "identf": np.eye(H, dtype=f32),
        "ones1": np.ones((1, H), f32),
        "invde": inv_de.reshape(E, 1),
    }

    in_maps = []
    for ci in range(NC):
        n0 = ci * NL
        xc = x[n0 : n0 + NL, :T, :].astype(f32)  # [NL, T, DF]
        xa = np.ones((7, T, NL), f32)
        xa[:6] = xc.transpose(2, 1, 0)
        hn = NL // 2
        xA = xa[:, :, :hn].reshape(7, T * hn)
        xB = xa[:, :, hn:].reshape(7, T * hn)
        x4 = np.zeros((128, T * hn), f32)
        x4[0:7] = xA
        x4[32:39] = xA
        x4[64:71] = xB
        x4[96:103] = xB
        ghc = GH[n0 : n0 + NL].astype(f32)  # [NL, E]
        gh_nm = ghc.reshape(NCH, 128, E).transpose(1, 0, 2).reshape(128, NCH * E)
        invdv_nm = inv_dv[n0 : n0 + NL].reshape(NCH, 128).T.copy()
        m = dict(shared)
        m["x"] = x4.astype(bf)
        m["gh"] = np.ascontiguousarray(gh_nm)
        m["invdv"] = np.ascontiguousarray(invdv_nm)
        in_maps.append(m)
    return in_maps


def kernel(**inputs):
    if "nc" not in _CACHE:
        _CACHE["nc"] = _build_program()
    nc = _CACHE["nc"]
    in_maps = _prep_inputs(**inputs)
    res = run_bass_kernel_spmd(nc, in_maps, list(range(NC)))
    out = np.concatenate([res.results[i]["y"][0] for i in range(NC)])
    return out.astype(np.float32)


def _install_profile_shim():
    """Recreate the antenv.axon_hooks NTFF profile hook missing from this image."""
    import types
    import ctypes
    import contextlib

    if "antenv.axon_hooks" in sys.modules:
        return
    so_path = "/opt/axon/libaxon_pjrt.so"
    lib = ctypes.CDLL(so_path)
    lib.axon_start_nrt_profile.argtypes = [
        ctypes.POINTER(ctypes.c_int64), ctypes.c_size_t,
    ]
    lib.axon_start_nrt_profile.restype = ctypes.c_int64
    lib.axon_stop_nrt_profile.argtypes = [ctypes.c_char_p]
    lib.axon_stop_nrt_profile.restype = ctypes.c_int64

    @contextlib.contextmanager
    def _hook(output_dir, device_ids):
        import jax

        jax.devices()
        if device_ids:
            ids = (ctypes.c_int64 * len(device_ids))(*device_ids)
            rc = lib.axon_start_nrt_profile(ids, len(device_ids))
        else:
            rc = lib.axon_start_nrt_profile(None, 0)
        if rc != 0:
            raise RuntimeError(f"axon_start_nrt_profile rc={rc}")
        try:
            yield
        finally:
            n = lib.axon_stop_nrt_profile(str(output_dir).encode())
            print(f"profile: {n} file(s) written to {output_dir}")

    mod = types.ModuleType("antenv.axon_hooks")
    mod.get_axon_ntff_profile_hook = lambda: _hook
    mod.set_axon_ntff_profile_hook = lambda h: None
    sys.modules["antenv.axon_hooks"] = mod
    import antenv

    antenv.axon_hooks = mod

    import concourse.bass_utils as bu

    bu.upload_artifacts = lambda tmpdir: f"local://{tmpdir}"


def run_traced(inputs, tmpdir=None):
    """test.py helper: run with NTFF tracing, return (output, BassKernelResults)."""
    _install_profile_shim()
    if "nc" not in _CACHE:
        _CACHE["nc"] = _build_program()
    nc = _CACHE["nc"]
    in_maps = _prep_inputs(**inputs)
    res = run_bass_kernel_spmd(
        nc, in_maps, list(range(NC)), trace=True, tmpdir=tmpdir
    )
    out = np.concatenate([res.results[i]["y"][0] for i in range(NC)])
    return out.astype(np.float32), res
